# revision 1
# baseline (speedup 1.0000x reference)
"""Trainium2 Bass kernel for nn_ContrastiveLoss (B=4096, D=1024, 8 cores).

loss = mean over [B,B] of
    labels*(1-sim0) + (1-labels)*relu(sim0-0.5)
  + labels*(1-sim1) + (1-labels)*relu(sim1-0.5)
where sim_k = cos_sim(fc_feats_k[i], textual_features[j]).

Strategy (data-parallel over rows, hint-conformant):
  * Each of the 8 cores gets a 512-row slice of fc_feats_0/1 and labels.
  * textual_features is row-sharded for the normalize+transpose work, then
    the normalized+transposed (bf16) [D, B] operand is AllGathered so every
    core holds the full tn^T for its matmuls.
  * Per core: S-tiles [128,1024] (sim0|sim1 halves) are produced in PSUM and
    consumed on-chip (never hit DRAM):
        total_core = 2*sum(L) + sum(relu(S-.5)) - sum(L ⊙ (S + relu(S-.5)))
    accumulated via ACT accum_out / DVE tensor_tensor_reduce.
  * Host sums the 8x[128] partials and divides by B*B.

Self-contained: hardcodes shapes; only needs the concourse package.
"""

import os
import sys

import numpy as np

B = 4096
D = 1024
NCORES = 8
ROWS = B // NCORES          # 512 rows of f0/f1/labels per core
IT = ROWS // 128            # 4 i-tiles per core
KS = D // 128               # 8 k-subtiles (contraction)
JC = B // 512               # 8 j-chunks of 512 columns
MARGIN = 0.5
EPS = 1e-8

_CACHE = {}

# 1 = share normalized/transposed textual features via AllGather collective;
# 0 = every core normalizes+transposes the full textual_features locally.
USE_COLLECTIVE = os.environ.get("KERNEL_USE_COLLECTIVE", "1") == "1"
# fp8 = e4m3 DoubleRow matmuls (2x PE throughput), bf16 = plain bf16.
MM_DTYPE = os.environ.get("KERNEL_MM_DTYPE", "fp8")
# fp8 operands are scaled by 64 (keeps values in e4m3 normal range);
# products carry 64*64, undone by the consumer's free scale slot.
FP8_SCALE = 64.0


def _import_concourse():
    try:
        import concourse.bass  # noqa: F401
    except ImportError:
        for p in ("/opt/trn_rl_repo", "/root/.axon_site/_ro/trn_rl_repo"):
            if os.path.isdir(p) and p not in sys.path:
                sys.path.insert(0, p)
        import concourse.bass  # noqa: F401


def _build_nc():
    """Build + schedule + compile the per-core Bass program (SPMD: same
    program on all 8 cores, different input slices)."""
    _import_concourse()
    import concourse.bass as bass
    import concourse.mybir as mybir
    import concourse.tile as tile
    from concourse import bacc
    from concourse.masks import make_identity

    f32 = mybir.dt.float32
    bf16 = mybir.dt.bfloat16
    AF = mybir.ActivationFunctionType
    OP = mybir.AluOpType
    AX = mybir.AxisListType

    fp8 = MM_DTYPE == "fp8"
    wdt = mybir.dt.float8e4 if fp8 else bf16      # matmul operand dtype
    op_scale = FP8_SCALE if fp8 else 1.0          # folded into 1/norm
    s_inv = 1.0 / (op_scale * op_scale)           # undo in consumers

    nc = bacc.Bacc(
        "TRN2",
        target_bir_lowering=False,
        debug=False,
        num_devices=NCORES,
    )

    tx_rows = ROWS if USE_COLLECTIVE else B
    f0_d = nc.dram_tensor("f0", [ROWS, D], f32, kind="ExternalInput").ap()
    f1_d = nc.dram_tensor("f1", [ROWS, D], f32, kind="ExternalInput").ap()
    tx_d = nc.dram_tensor("tx", [tx_rows, D], f32, kind="ExternalInput").ap()
    lab_d = nc.dram_tensor("lab", [ROWS, B], f32, kind="ExternalInput").ap()
    out_d = nc.dram_tensor("outv", [128, 1], f32, kind="ExternalOutput").ap()

    with tile.TileContext(nc) as tc:
        with (
            tc.tile_pool(name="constp", bufs=1) as constp,
            tc.tile_pool(name="stage", bufs=2) as stage,
            tc.tile_pool(name="small", bufs=4) as small,
            tc.tile_pool(name="wT", bufs=1) as wTp,
            tc.tile_pool(name="tnTp", bufs=1) as tnTp,
            tc.tile_pool(name="labp", bufs=3) as labp,
            tc.tile_pool(name="lscrp", bufs=1) as lscrp,
            tc.tile_pool(name="actsp", bufs=2) as actsp,
            tc.tile_pool(name="accp", bufs=1) as accp,
            tc.tile_pool(name="tpsum", bufs=2, space="PSUM") as tpsum,
            tc.tile_pool(name="mpsum", bufs=3, space="PSUM") as mpsum,
            tc.tile_pool(name="dram", bufs=1, space="DRAM") as dram,
        ):
            ident = constp.tile([128, 128], bf16)
            make_identity(nc, ident)
            negmargin = constp.tile([128, 1], f32)
            nc.gpsimd.memset(negmargin, -MARGIN)

            # ---- accumulators (each column written exactly once) ----
            racc = accp.tile([128, IT * JC], f32)      # sum relu(S-.5), per pair
            pacc = accp.tile([128, IT * JC], f32)      # sum L*(S+relu), per pair
            lacc = accp.tile([128, 2 * IT], f32)       # sum L, per half-i-tile

            def normalize_rows(src_ap, it, out_dtype):
                """Load [128, D] tile, return SBUF tile of row-normalized
                values (cast to out_dtype)."""
                nat = stage.tile([128, D], f32, tag="nat", name=f"nat_{it}")
                nc.sync.dma_start(nat, src_ap[it * 128:(it + 1) * 128, :])
                sq = stage.tile([128, D], f32, tag="sq", name=f"sq_{it}")
                ssq = small.tile([128, 1], f32, tag="ssq", name=f"ssq_{it}")
                nc.scalar.activation(sq, nat, AF.Square, accum_out=ssq)
                nrm = small.tile([128, 1], f32, tag="nrm", name=f"nrm_{it}")
                nc.scalar.activation(nrm, ssq, AF.Sqrt)
                nc.vector.tensor_scalar_max(nrm, nrm, EPS)
                rin = small.tile([128, 1], f32, tag="rin", name=f"rin_{it}")
                nc.vector.reciprocal(rin, nrm)
                if op_scale != 1.0:
                    nc.vector.tensor_scalar_mul(rin, rin, op_scale)
                nrmd = stage.tile([128, D], out_dtype, tag="nrmd", name=f"nrmd_{it}")
                nc.vector.tensor_scalar_mul(nrmd, nat, rin)
                return nrmd

            def transpose_into(nrmd, dst_T, it, copy_engine):
                """PE-transpose [128, D] normalized tile into dst_T[:, ks,
                it*128:(it+1)*128] (layout [d_part, ks, row])."""
                for ks in range(KS):
                    pst = tpsum.tile([128, 128], bf16, tag="pst",
                                     name=f"pst_{it}_{ks}")
                    nc.tensor.transpose(pst, nrmd[:, ks * 128:(ks + 1) * 128],
                                        ident)
                    copy_engine(dst_T[:, ks, it * 128:(it + 1) * 128], pst)

            tnT = tnTp.tile([128, JC * KS, 512], wdt)

            if USE_COLLECTIVE:
                # ---- phase A: normalize+transpose own t-slice, share ----
                tT_loc = wTp.tile([128, KS, ROWS], wdt)
                for it in range(IT):
                    nrmd = normalize_rows(tx_d, it, bf16)
                    transpose_into(nrmd, tT_loc, it,
                                   lambda o, i: nc.vector.tensor_copy(o, i))

                tT_loc_d = dram.tile([KS, 128, ROWS], wdt, name="tT_loc_d")
                nc.sync.dma_start(tT_loc_d.rearrange("ks p j -> p ks j"),
                                  tT_loc)
                tT_all_d = dram.tile([NCORES, KS, 128, ROWS], wdt,
                                     addr_space="Shared", name="tT_all_d")
                nc.gpsimd.collective_compute(
                    "AllGather",
                    OP.bypass,
                    replica_groups=[list(range(NCORES))],
                    ins=[tT_loc_d.opt()],
                    outs=[tT_all_d.opt()],
                )

            # ---- phase A2: normalize + transpose own f0/f1 slices ----
            # Alternate evacuation copies between DVE and ACT for balance.
            def _mixed_copy(o, i, ks=[0]):
                ks[0] ^= 1
                (nc.vector.tensor_copy if ks[0] else nc.scalar.copy)(o, i)

            f0T = wTp.tile([128, KS, ROWS], wdt)
            f1T = wTp.tile([128, KS, ROWS], wdt)
            for it in range(IT):
                nrmd = normalize_rows(f0_d, it, bf16)
                transpose_into(nrmd, f0T, it, _mixed_copy)
            for it in range(IT):
                nrmd = normalize_rows(f1_d, it, bf16)
                transpose_into(nrmd, f1T, it, _mixed_copy)

            # ---- labels load + sum(L) fill the collective wait ----
            Lts = []
            for ic in range(IT):
                Lt = labp.tile([128, B], f32, tag="Lt", name=f"Lt_{ic}")
                nc.sync.dma_start(Lt, lab_d[ic * 128:(ic + 1) * 128, :])
                for h in range(2):
                    lscr = lscrp.tile([128, B // 2], bf16, tag="lscr",
                                      name=f"lscr_{ic}_{h}")
                    nc.scalar.activation(
                        lscr, Lt[:, h * (B // 2):(h + 1) * (B // 2)],
                        AF.Copy, accum_out=lacc[:, 2 * ic + h:2 * ic + h + 1])
                Lts.append(Lt)

            if USE_COLLECTIVE:
                # ---- load gathered tn^T: [128, JC*KS, 512] ----
                for jc in range(JC):
                    nc.sync.dma_start(
                        tnT[:, jc * KS:(jc + 1) * KS, :],
                        tT_all_d[jc].rearrange("ks p j -> p ks j"),
                    )
            else:
                # Every core normalizes + transposes the FULL t locally.
                for it in range(B // 128):
                    nrmd = normalize_rows(tx_d, it, bf16)
                    jc, joff = it // IT, (it % IT) * 128
                    for ks in range(KS):
                        pst = tpsum.tile([128, 128], bf16, tag="pst",
                                         name=f"tpst_{it}_{ks}")
                        nc.tensor.transpose(
                            pst, nrmd[:, ks * 128:(ks + 1) * 128], ident)
                        dst = tnT[:, jc * KS + ks, joff:joff + 128]
                        if ks % 2 == 0:
                            nc.scalar.copy(dst, pst)
                        else:
                            nc.vector.tensor_copy(dst, pst)

            # ---- phase B: matmuls + fused loss ----
            for ic in range(IT):
                Lt = Lts[ic]
                for jc in range(JC):
                    pi = ic * JC + jc
                    # S-tile: [:, :512] = sim0, [:, 512:] = sim1
                    ps = mpsum.tile([128, 1024], f32, tag="ps",
                                    name=f"ps_{ic}_{jc}")
                    isl = slice(ic * 128, (ic + 1) * 128)
                    if fp8:
                        DR = mybir.MatmulPerfMode.DoubleRow
                        for k2 in range(KS // 2):
                            ksl = slice(jc * KS + 2 * k2, jc * KS + 2 * k2 + 2)
                            wsl = slice(2 * k2, 2 * k2 + 2)
                            nc.tensor.matmul(
                                ps[:, 0:512], f0T[:, wsl, isl],
                                tnT[:, ksl, :], perf_mode=DR,
                                start=(k2 == 0), stop=(k2 == KS // 2 - 1),
                            )
                        for k2 in range(KS // 2):
                            ksl = slice(jc * KS + 2 * k2, jc * KS + 2 * k2 + 2)
                            wsl = slice(2 * k2, 2 * k2 + 2)
                            nc.tensor.matmul(
                                ps[:, 512:1024], f1T[:, wsl, isl],
                                tnT[:, ksl, :], perf_mode=DR,
                                start=(k2 == 0), stop=(k2 == KS // 2 - 1),
                            )
                    else:
                        for ks in range(KS):
                            nc.tensor.matmul(
                                ps[:, 0:512], f0T[:, ks, isl],
                                tnT[:, jc * KS + ks, :],
                                start=(ks == 0), stop=(ks == KS - 1),
                            )
                        for ks in range(KS):
                            nc.tensor.matmul(
                                ps[:, 512:1024], f1T[:, ks, isl],
                                tnT[:, jc * KS + ks, :],
                                start=(ks == 0), stop=(ks == KS - 1),
                            )
                    # r = relu(S*s_inv - margin); accumulate sum(r)
                    r01 = actsp.tile([128, 1024], bf16, tag="r01",
                                     name=f"r01_{ic}_{jc}")
                    nc.scalar.activation(r01, ps, AF.Relu, bias=negmargin,
                                         scale=s_inv,
                                         accum_out=racc[:, pi:pi + 1])
                    # uv = S*s_inv + r  (w0 | w1)
                    uv = actsp.tile([128, 1024], bf16, tag="uv",
                                    name=f"uv_{ic}_{jc}")
                    nc.vector.scalar_tensor_tensor(
                        out=uv, in0=ps, scalar=s_inv, in1=r01,
                        op0=OP.mult, op1=OP.add)
                    # w = w0 + w1
                    w = actsp.tile([128, 512], bf16, tag="w",
                                   name=f"w_{ic}_{jc}")
                    nc.vector.tensor_add(w, uv[:, 0:512], uv[:, 512:1024])
                    # sum(L * w) into pacc (fused multiply+reduce)
                    scr = actsp.tile([128, 512], bf16, tag="scr",
                                     name=f"scr_{ic}_{jc}")
                    nc.vector.scalar_tensor_tensor(
                        out=scr,
                        in0=Lt[:, jc * 512:(jc + 1) * 512],
                        scalar=1.0,
                        in1=w,
                        op0=OP.bypass,
                        op1=OP.mult,
                        accum_out=pacc[:, pi:pi + 1],
                    )

            # ---- finisher: out = 2*sum(L) + sum(r) - sum(L*w) ----
            ra = small.tile([128, 1], f32, tag="fin", name="ra")
            nc.vector.reduce_sum(ra, racc, axis=AX.X)
            pa = small.tile([128, 1], f32, tag="fin", name="pa")
            nc.vector.reduce_sum(pa, pacc, axis=AX.X)
            la = small.tile([128, 1], f32, tag="fin", name="la")
            nc.vector.reduce_sum(la, lacc, axis=AX.X)
            tmp = small.tile([128, 1], f32, tag="fin", name="tmp")
            nc.vector.scalar_tensor_tensor(
                out=tmp, in0=la, scalar=2.0, in1=ra,
                op0=OP.mult, op1=OP.add,
            )
            ov = small.tile([128, 1], f32, tag="fin", name="ov")
            nc.vector.tensor_sub(ov, tmp, pa)
            nc.sync.dma_start(out_d, ov)

    nc.compile()
    return nc


def _get_nc():
    if "nc" not in _CACHE:
        _CACHE["nc"] = _build_nc()
    return _CACHE["nc"]


def _make_in_maps(fc_feats_0, fc_feats_1, textual_features, labels):
    in_maps = []
    tx_full = np.ascontiguousarray(textual_features, dtype=np.float32)
    for c in range(NCORES):
        sl = slice(c * ROWS, (c + 1) * ROWS)
        in_maps.append({
            "f0": np.ascontiguousarray(fc_feats_0[sl], dtype=np.float32),
            "f1": np.ascontiguousarray(fc_feats_1[sl], dtype=np.float32),
            "tx": tx_full[sl] if USE_COLLECTIVE else tx_full,
            "lab": np.ascontiguousarray(labels[sl], dtype=np.float32),
        })
    return in_maps


def run(fc_feats_0, fc_feats_1, textual_features, labels, trace=False):
    """Run on 8 NeuronCores; returns (loss_scalar, BassKernelResults)."""
    _import_concourse()
    from concourse.bass_utils import run_bass_kernel_spmd

    nc = _get_nc()
    in_maps = _make_in_maps(np.asarray(fc_feats_0), np.asarray(fc_feats_1),
                            np.asarray(textual_features), np.asarray(labels))
    res = run_bass_kernel_spmd(nc, in_maps, list(range(NCORES)), trace=trace)
    total = 0.0
    for c in range(NCORES):
        total += float(np.asarray(res.results[c]["outv"], dtype=np.float64).sum())
    loss = total / float(B * B)
    return np.asarray(loss, dtype=np.float32), res


def kernel(fc_feats_0, fc_feats_1, textual_features, labels):
    loss, _ = run(fc_feats_0, fc_feats_1, textual_features, labels, trace=False)
    return loss

